# revision 32
# baseline (speedup 1.0000x reference)
"""Causal self-attention (B=4, T=2048, D=1024, H=16) on 8 NeuronCores.

Sharding: core c handles batch b=c//2 and head-group hg=c%2 (8 of 16 heads).
Per core: column-parallel Wq/Wk/Wv (512 cols), row-parallel Wo (512 rows).
Host sums the two partial outputs per batch and adds bo_eff. No collectives.

Structure (v4):
  - x^T resident in SBUF (bf16), loaded once with block DMAs (sync queue).
  - bk drops (softmax invariance).  bv folds into bo on host.  bq is added
    during the Q-projection PSUM->SBUF copy as a per-partition scalar
    (tensor_scalar_add), so V needs no scaling and exp needs no bias.
  - V phase: 8 matmuls/tile + one PSUM->SBUF copy (alternating ACT/DVE) +
    a constant ones column (sumexp rides the PV matmul at M=65).
  - Scores: both heads write ONE [128,1024] 2-bank PSUM tile; one exp per
    step covers both heads; causal triangle zeroed post-exp on DVE.
  - Division per q-block: d rows -> [2,512]; one reciprocal_approx_fast;
    1/d broadcast to 128 partitions via gpsimd partition_broadcast (f32);
    one DVE multiply into oat.  bc selector matmuls eliminated.
  - Fine-grained PE fillers: projection / O-projection work is emitted in
    2-matmul quanta between attention steps so the ACT exp pipeline never
    starves and the PE never idles (idle resets the PE clock ramp).
  - O-proj interleaves into pair 3 per-chunk; final tiles split copies
    across ACT/DVE and y DMAs across gpsimd/sync queues.
"""

import os
from collections import deque
from contextlib import ExitStack

import ml_dtypes
import numpy as np

import concourse.bacc as bacc
import concourse.mybir as mybir
import concourse.tile as tile
from concourse.bass_utils import run_bass_kernel_spmd

B, T, D, H, DK = 4, 2048, 1024, 16, 64
HL = 8  # heads per core
CD = HL * DK  # 512 local channels
NP = 128  # partitions
QB = 512  # query block
NDC = D // NP  # 8 din chunks
NTT = T // NP  # 16 t-tiles
NTB = T // QB  # 4 t-blocks
NPAIR = HL // 2  # 4 head pairs
VW = DK + 2  # v row stride (64 data + 1 ones + 1 pad)
F32 = mybir.dt.float32
BF16 = mybir.dt.bfloat16
Exp = mybir.ActivationFunctionType.Exp
Identity = mybir.ActivationFunctionType.Identity

USE_PBCAST = False  # gpsimd partition_broadcast for 1/d; else selector matmul

_CACHE: dict = {}


def _build_nc():
    nc = bacc.Bacc("TRN2", target_bir_lowering=False, debug=False)
    xt = nc.dram_tensor("xt", [D, T], BF16, kind="ExternalInput")
    wq = nc.dram_tensor("wq", [D, CD], BF16, kind="ExternalInput")
    wk = nc.dram_tensor("wk", [D, CD], BF16, kind="ExternalInput")
    wv = nc.dram_tensor("wv", [D, CD], BF16, kind="ExternalInput")
    bqv = nc.dram_tensor("bqv", [NP, NPAIR], F32, kind="ExternalInput")
    wo = nc.dram_tensor("wo", [CD, D], BF16, kind="ExternalInput")
    tri = nc.dram_tensor("tri", [NP, NP], BF16, kind="ExternalInput")
    sel2d = None
    if not USE_PBCAST:
        sel2d = nc.dram_tensor("sel2d", [2, NP], BF16, kind="ExternalInput")
    y = nc.dram_tensor("y", [T, D], BF16, kind="ExternalOutput")

    with tile.TileContext(nc) as tc, ExitStack() as ctx:
        _body(nc, tc, ctx, xt, wq, wk, wv, bqv, wo, tri, sel2d, y)
    nc.compile()
    return nc


def _body(nc, tc, ctx, xt, wq, wk, wv, bqv, wo, tri, sel2d, y):
    const = ctx.enter_context(tc.tile_pool(name="const", bufs=1))
    xtp = ctx.enter_context(tc.tile_pool(name="xt", bufs=1))
    vpool = ctx.enter_context(tc.tile_pool(name="v", bufs=1))
    oatp = ctx.enter_context(tc.tile_pool(name="oat", bufs=1))
    wqkp = ctx.enter_context(tc.tile_pool(name="wqk", bufs=2))
    qkp = ctx.enter_context(tc.tile_pool(name="qk", bufs=2))
    etp = ctx.enter_context(tc.tile_pool(name="et", bufs=2))
    pvsp = ctx.enter_context(tc.tile_pool(name="pvs", bufs=2))
    dnp = ctx.enter_context(tc.tile_pool(name="dn", bufs=2))
    bcp = ctx.enter_context(tc.tile_pool(name="bc", bufs=2))
    smallp = ctx.enter_context(tc.tile_pool(name="small", bufs=2))
    wop = ctx.enter_context(tc.tile_pool(name="wop", bufs=1))
    wvp = ctx.enter_context(tc.tile_pool(name="wvp", bufs=1))
    # PSUM: proj(2 banks) + scores(2x2) + pv(2x1) = 8 banks
    projps = ctx.enter_context(tc.tile_pool(name="projps", bufs=2, space="PSUM"))

    # ---- constants (gpsimd queue: keep the scalar queue free for wv) ----
    tri_sb = const.tile([NP, NP], BF16, tag="tri")
    nc.gpsimd.dma_start(tri_sb[:], tri[:])
    bqv_sb = const.tile([NP, NPAIR], F32, tag="bqv")
    nc.gpsimd.dma_start(bqv_sb[:], bqv[:])
    sel2 = None
    if not USE_PBCAST:
        sel2 = [
            const.tile([1, NP], BF16, tag=f"sel2_{h}", name=f"sel2_{h}")
            for h in range(2)
        ]
        for h in range(2):
            nc.gpsimd.dma_start(sel2[h][:], sel2d[h : h + 1, :])
    warm = const.tile([1, 2], F32, tag="warm")
    nc.vector.memset(warm[:], 0.0)
    nc.scalar.activation(warm[:], warm[:], Exp)

    # ---- V weights first, split across both HWDGE queues ----
    wv_sb = wvp.tile([NP, NDC, CD], BF16, tag="wv")
    for d in range(NDC):
        q = nc.scalar if d % 2 == 0 else nc.sync
        q.dma_start(wv_sb[:, d, :], wv[d * NP : (d + 1) * NP, :])

    # ---- resident x^T (bf16): block DMAs, t-block major so V starts early
    xt_sb = xtp.tile([NP, NDC, T], BF16, tag="xt")
    for tb in range(NTB):
        bsl = slice(tb * QB, (tb + 1) * QB)
        for d in range(NDC):
            nc.sync.dma_start(xt_sb[:, d, bsl], xt[d * NP : (d + 1) * NP, bsl])

    wq_t = [None] * NPAIR
    wk_t = [None] * NPAIR

    def emit_wqk_dma(c):
        wq_t[c] = wqkp.tile([NP, NDC, NP], BF16, tag="wqc", name="wqc")
        wk_t[c] = wqkp.tile([NP, NDC, NP], BF16, tag="wkc", name="wkc")
        for d in range(NDC):
            nc.gpsimd.dma_start(
                wq_t[c][:, d, :], wq[d * NP : (d + 1) * NP, c * NP : (c + 1) * NP]
            )
            nc.gpsimd.dma_start(
                wk_t[c][:, d, :], wk[d * NP : (d + 1) * NP, c * NP : (c + 1) * NP]
            )

    emit_wqk_dma(0)

    # ---- V tiles: ones column constant, data filled per t-tile ----
    v_sb = [
        vpool.tile([NP, HL, VW], BF16, tag=f"v{tt}", name=f"v{tt}")
        for tt in range(NTT)
    ]
    for tt in range(NTT):
        nc.vector.memset(v_sb[tt][:, :, DK : DK + 1], 1.0)

    # ---- V phase: plain projection + PSUM->SBUF copy ----
    for tt in range(NTT):
        tsl = slice(tt * NP, (tt + 1) * NP)
        psv = projps.tile([NP, CD], F32, tag="proj")
        for d in range(NDC):
            nc.tensor.matmul(
                psv[:], xt_sb[:, d, tsl], wv_sb[:, d, :],
                start=(d == 0), stop=(d == NDC - 1),
            )
        dst = v_sb[tt][:, :, 0:DK]
        src = psv[:].rearrange("p (h d) -> p h d", h=HL)
        if tt % 2 == 0:
            nc.scalar.activation(dst, src, Identity)
        else:
            nc.vector.tensor_copy(dst, src)

    # wo loaded on the (idle) sync queue before pair-2 attention
    wo_sb = wop.tile([NP, NPAIR, D], BF16, tag="wo")

    attnps_cm = tc.tile_pool(name="attnps", bufs=1, space="PSUM")
    attnps = attnps_cm.__enter__()

    def score_tile():
        return attnps.tile([NP, 2 * QB], F32, tag="s", bufs=2, name="s")

    def pv_tile(h):
        return attnps.tile([DK + 1, QB], F32, tag=f"pv{h}", bufs=1, name=f"pv{h}")

    # ---- Q/K projection quanta: 2 matmuls (or 2 copies) per yield ----
    qt_t = [None] * NPAIR
    kt_t = [None] * NPAIR

    def alloc_qk(c):
        qt_t[c] = qkp.tile([NP, T], BF16, tag="qt", name=f"qt{c}")
        kt_t[c] = qkp.tile([NP, T], BF16, tag="kt", name=f"kt{c}")

    def proj_quanta(c):
        for tbp in range(NTB // 2):
            for which in ("q", "k"):
                slA = slice(2 * tbp * QB, (2 * tbp + 1) * QB)
                slB = slice((2 * tbp + 1) * QB, (2 * tbp + 2) * QB)
                w_sb = wq_t[c] if which == "q" else wk_t[c]
                dst = qt_t[c] if which == "q" else kt_t[c]
                psA = projps.tile([NP, QB], F32, tag="proj", name="psA")
                psB = projps.tile([NP, QB], F32, tag="proj", name="psB")
                for d in range(NDC):
                    nc.tensor.matmul(
                        psA[:], w_sb[:, d, :], xt_sb[:, d, slA],
                        start=(d == 0), stop=(d == NDC - 1),
                    )
                    nc.tensor.matmul(
                        psB[:], w_sb[:, d, :], xt_sb[:, d, slB],
                        start=(d == 0), stop=(d == NDC - 1),
                    )
                    yield
                if which == "q":
                    nc.vector.tensor_scalar_add(dst[:, slA], psA[:], bqv_sb[:, c : c + 1])
                    nc.vector.tensor_scalar_add(dst[:, slB], psB[:], bqv_sb[:, c : c + 1])
                else:
                    nc.vector.tensor_copy(dst[:, slA], psA[:])
                    nc.vector.tensor_copy(dst[:, slB], psB[:])
                yield

    # ---- O-proj quanta ----
    oat = [
        oatp.tile([NP, T], BF16, tag=f"oat{c}", name=f"oat{c}") for c in range(NPAIR)
    ]
    ndma = [0]

    def oproj_quanta(tt, eng):
        tsl = slice(tt * NP, (tt + 1) * NP)
        ya = projps.tile([NP, QB], F32, tag="proj", name="ya")
        yb = projps.tile([NP, QB], F32, tag="proj", name="yb")
        for cc in range(NPAIR):
            nc.tensor.matmul(
                ya[:], oat[cc][:, tsl], wo_sb[:, cc, 0:QB],
                start=(cc == 0), stop=(cc == NPAIR - 1),
            )
            nc.tensor.matmul(
                yb[:], oat[cc][:, tsl], wo_sb[:, cc, QB:D],
                start=(cc == 0), stop=(cc == NPAIR - 1),
            )
            yield
        for dh, yps in enumerate((ya, yb)):
            yst = smallp.tile([NP, QB], BF16, tag="ystage", bufs=4, name="yst")
            if eng == "mixed":
                e = "scalar" if dh == 0 else "vector"
            else:
                e = eng
            if e == "scalar":
                nc.scalar.activation(yst[:], yps[:], Identity)
            else:
                nc.vector.tensor_copy(yst[:], yps[:])
            q = nc.gpsimd if ndma[0] % 2 == 0 else nc.sync
            ndma[0] += 1
            q.dma_start(y[tsl, dh * QB : (dh + 1) * QB], yst[:])
        yield

    # ---- division tail: extract d, reciprocal, broadcast 1/d, scale oat ----
    # All DVE work is deferred into the filler stream so it lands BEHIND the
    # next pair's critical qt/kt copies on the DVE queue, not ahead of them.
    def div_tail(c, qb, pv, pvs, chunked=False):
        qsl0 = qb * QB
        dscr = [None, None]
        for h in range(2):
            dcp = dnp.tile([1, QB], F32, tag=f"dcp{h}", name=f"dcp{h}")
            nc.vector.tensor_copy(dcp[:], pv[h][DK : DK + 1, :])
            dscr[h] = dnp.tile([1, QB], F32, tag=f"dscr{h}", name=f"dscr{h}")
            nc.vector.reciprocal_approx_fast(dscr[h][:], dcp[:])
            yield
        recbf = [None, None]
        for h in range(2):
            recbf[h] = dnp.tile([1, QB], BF16, tag=f"recbf{h}", name=f"recbf{h}")
            nc.vector.tensor_copy(recbf[h][:], dscr[h][:])
        yield
        yield  # spacing: let the DVE chain drain so the bc matmul
        yield  # below never stalls the in-order PE queue
        yield
        yield
        bc = projps.tile([NP, QB], F32, tag="proj", name="bc")
        for h in range(2):
            nc.tensor.matmul(
                bc[:], sel2[h][:], recbf[h][:],
                start=(h == 0), stop=(h == 1),
            )
        yield
        if chunked:
            # column-chunked so each final O-proj tile unblocks ASAP
            for j in range(4):
                cs = slice(j * NP, (j + 1) * NP)
                nc.vector.tensor_mul(
                    oat[c][:, qsl0 + j * NP : qsl0 + (j + 1) * NP],
                    pvs[:, qb, cs], bc[:, cs],
                )
                yield
        else:
            nc.vector.tensor_mul(
                oat[c][:, qsl0 : qsl0 + QB], pvs[:, qb, :], bc[:]
            )
            yield

    # fillers: deque of (generator, on_done_callback)
    fillers = deque()

    def consume(budget):
        done = 0
        while done < budget and fillers:
            g, cb = fillers[0]
            try:
                next(g)
                done += 1
            except StopIteration:
                fillers.popleft()
                if cb is not None:
                    cb()
        return done

    # pair 0 projections up front
    alloc_qk(0)
    for _ in proj_quanta(0):
        pass

    oproj_emitted = [0]

    # ---- attention per pair ----
    carry = None
    for c in range(NPAIR):
        if c + 1 < NPAIR:
            emit_wqk_dma(c + 1)
            alloc_qk(c + 1)
            fillers.append((proj_quanta(c + 1), None))
        if carry is not None:
            fillers.appendleft(carry)
            carry = None
        if c == 2:
            for cc in range(NPAIR):
                nc.sync.dma_start(wo_sb[:, cc, :], wo[cc * NP : (cc + 1) * NP, :])
        qt, kt = qt_t[c], kt_t[c]
        kt_total = sum(4 * qb + 4 for qb in range(NTB))  # 40
        kt_seen = 0
        nflr = 0
        # finish fillers ~6 steps before pair end so the next pair's qt/kt
        # copies are never stuck behind late filler work on the DVE queue
        fill_total = 58 if c < NPAIR - 1 else 200
        fill_den = kt_total - 6

        pvs = pvsp.tile([NP, NTB, QB], BF16, tag="pvs", name=f"pvs{c}")
        consume(3)  # seed the PE pipeline across the pair boundary

        def make_oproj_adder(lo, hi, eng="vector"):
            def add():
                for tt in range(lo, hi):
                    fillers.append((oproj_quanta(tt, eng), None))
                    oproj_emitted[0] += 1
            return add

        for qb in range(NTB):
            qsl0 = qb * QB
            nkt = 4 * qb + 4
            pv = None  # allocated lazily AFTER step-0 fillers (which contain
            # the previous q-block's pv readers) to keep WAR deps correct
            prev = None
            for kti in range(nkt):
                di = kti - 4 * qb
                o = max(di, 0) * NP
                sps = score_tile()
                for h in range(2):
                    # head 1 packs left: [QB : 2QB-o] so the exp region is
                    # contiguous ([o : 2QB-o]) and o columns shorter
                    lo = o if h == 0 else QB
                    nc.tensor.matmul(
                        sps[:, lo : lo + QB - o],
                        kt[64 * h : 64 * h + 64, kti * NP : (kti + 1) * NP],
                        qt[64 * h : 64 * h + 64, qsl0 + o : qsl0 + QB],
                        start=True, stop=True,
                        tile_position=(64 * h, 0),
                    )
                kt_seen += 1
                want = (kt_seen * fill_total) // fill_den
                nflr += consume(max(0, want - nflr))
                if prev is not None:
                    if pv is None:
                        pv = [pv_tile(h) for h in range(2)]
                    _emit_exp_pv(nc, prev, qb, etp, tri_sb, pv, nkt, v_sb, c)
                prev = (kti, o, sps)
            if pv is None:
                pv = [pv_tile(h) for h in range(2)]
            _emit_exp_pv(nc, prev, qb, etp, tri_sb, pv, nkt, v_sb, c)

            # extract unnormalized out^T; division is deferred into fillers
            for h in range(2):
                nc.vector.tensor_copy(pvs[64 * h : 64 * h + 64, qb, :], pv[h][0:DK, :])
            last = c == NPAIR - 1 and qb == NTB - 1
            cb = None
            if c == NPAIR - 1:
                cb = make_oproj_adder(
                    4 * qb, 4 * qb + 4, "mixed" if last else "vector"
                )
            g = div_tail(c, qb, pv, pvs, chunked=last)
            if last:
                # endgame: drain division (chunked) while final O-proj tiles
                # pipeline in behind each unblocked chunk
                fillers.appendleft((g, cb))
            elif qb == NTB - 1:
                carry = (g, cb)  # ride into the next pair's filler stream
            else:
                fillers.appendleft((g, cb))

        if c < NPAIR - 1:
            consume(1000000)

    consume(1000000)

    attnps_cm.__exit__(None, None, None)


def _emit_exp_pv(nc, prev, qb, etp, tri_sb, pv, nkt, v_sb, c):
    """one exp over both heads -> (triangle zero) -> 2 PV accumulates."""
    kti, o, sps = prev
    diag = kti >= 4 * qb
    et = etp.tile([NP, 2 * QB], BF16, tag="et", name="et")
    nc.scalar.activation(
        et[:, o : 2 * QB - o], sps[:, o : 2 * QB - o], Exp, scale=0.125
    )
    if diag:
        for h in range(2):
            lo = o if h == 0 else QB
            nc.vector.tensor_mul(
                et[:, lo : lo + NP], et[:, lo : lo + NP], tri_sb[:]
            )
    for h in range(2):
        lo = o if h == 0 else QB
        nc.tensor.matmul(
            pv[h][:, o:QB],
            v_sb[kti][:, 2 * c + h, 0 : DK + 1],
            et[:, lo : lo + QB - o],
            start=(kti == 0), stop=(kti == nkt - 1),
        )


def _install_ntff_hook_shim():
    """Provide the missing axon_hooks module so trace=True works under axon."""
    try:
        import sys
        import types

        if "antenv.axon_hooks" not in sys.modules:
            mod = types.ModuleType("antenv.axon_hooks")
            mod._hook = None
            mod.set_axon_ntff_profile_hook = lambda h: setattr(mod, "_hook", h)
            mod.get_axon_ntff_profile_hook = lambda: mod._hook
            sys.modules["antenv.axon_hooks"] = mod
            import antenv

            antenv.axon_hooks = mod
        from antenv.axon_hooks import (
            get_axon_ntff_profile_hook,
            set_axon_ntff_profile_hook,
        )

        if get_axon_ntff_profile_hook() is None:
            from trn_agent_boot.trn_boot import _ntff_profile_via_ctypes

            hook = _ntff_profile_via_ctypes("/opt/axon/libaxon_pjrt.so")
            if hook is not None:
                set_axon_ntff_profile_hook(hook)
    except Exception as e:  # noqa: BLE001
        print(f"ntff hook shim failed ({e}); running without trace")


def _bf(a: np.ndarray) -> np.ndarray:
    return np.ascontiguousarray(a, dtype=np.float32).astype(ml_dtypes.bfloat16)


def make_in_maps(x, Wq, bq, Wk, Wv, Wo):
    kk = np.arange(NP)[:, None]
    qq = np.arange(NP)[None, :]
    tri_np = (qq >= kk).astype(np.float32)
    in_maps = []
    for core in range(8):
        b, hg = core // 2, core % 2
        cs = slice(hg * CD, (hg + 1) * CD)
        bqv_np = np.ascontiguousarray(
            bq[cs].reshape(NPAIR, NP).T, dtype=np.float32
        )
        in_maps.append(
            {
                "xt": _bf(x[b].T),
                "wq": _bf(Wq[:, cs]),
                "wk": _bf(Wk[:, cs]),
                "wv": _bf(Wv[:, cs]),
                "bqv": bqv_np,
                "wo": _bf(Wo[cs, :]),
                "tri": _bf(tri_np),
            }
        )
        if not USE_PBCAST:
            sel2_np = np.zeros((2, NP), np.float32)
            sel2_np[0, 0:64] = 1.0
            sel2_np[1, 64:128] = 1.0
            in_maps[-1]["sel2d"] = _bf(sel2_np)
    return in_maps


def kernel(x, Wq, bq, Wk, bk, Wv, bv, Wo, bo):
    x = np.ascontiguousarray(np.asarray(x, dtype=np.float32))
    Wq, bq = np.asarray(Wq, np.float32), np.asarray(bq, np.float32)
    Wk = np.asarray(Wk, np.float32)
    Wv, bv = np.asarray(Wv, np.float32), np.asarray(bv, np.float32)
    Wo, bo = np.asarray(Wo, np.float32), np.asarray(bo, np.float32)

    if "nc" not in _CACHE:
        _CACHE["nc"] = _build_nc()
    nc = _CACHE["nc"]

    in_maps = make_in_maps(x, Wq, bq, Wk, Wv, Wo)

    trace = bool(os.environ.get("KERNEL_TRACE"))
    if trace:
        _install_ntff_hook_shim()
    res = run_bass_kernel_spmd(nc, in_maps, core_ids=list(range(8)), trace=trace)
    _CACHE["last_results"] = res

    bo_eff = bo + bv @ Wo
    out = np.empty((B, T, D), dtype=np.float32)
    for b in range(B):
        out[b] = (
            res.results[2 * b]["y"].astype(np.float32)
            + res.results[2 * b + 1]["y"].astype(np.float32)
            + bo_eff
        )
    return out
